# revision 8
# baseline (speedup 1.0000x reference)
"""v7: DVE+ACT only (gpsimd contends with DVE for SBUF ports - avoid),
per-scale PSUM accumulators, software-pipelined emission.

Math per pixel: d = x1-x0, u = Exp(d), sp = Ln(u+1), spm = sp-d,
v = Exp(-sp), g = 1-v, s2 = g*g, om2 = v*v,
loss = sum_s wt_s * (t0*sp*s2 + t1*spm*om2)   (wt applied on host).

Lessons baked in (v5/v6 traces):
- gpsimd TENSOR_TENSOR not only runs ~2ns/elem on bf16, it also slows
  concurrent DVE ops 2-3x (shared SBUF ports) -> all elementwise math
  on DVE (tensor_tensor 0.6ns/elem) + ACT (0.94ns/elem).
- STT runs at 1x; with per-scale PSUM accs the wt moves to the host
  and ap/am become 2x tensor_tensor.
- 4 groups: s1-merged [128,1024], s0 per sample [128,2048] x2,
  s2 [128,256] last (short tail). d-subs emitted interleaved so no
  engine head-of-line blocks on a late DMA.
- DMA order: small logits, x0_0, t_0, x0_1, t_1, x2 - targets are only
  needed by the PE masks, which run late anyway.
"""

import os
from contextlib import ExitStack

import numpy as np
import ml_dtypes

import concourse.bacc as bacc
import concourse.bass as bass
import concourse.mybir as mybir
import concourse.tile as tile
from concourse.bass_utils import run_bass_kernel_spmd

F32 = mybir.dt.float32
BF16 = mybir.dt.float16  # fp16: ACT writes 2-byte faster than bf16, more mantissa
AFT = mybir.ActivationFunctionType
ALU = mybir.AluOpType

N_CORES = 8
B, C, H, W = 16, 2, 512, 512
B_LOCAL = B // N_CORES  # 2
SCALE_WTS = (1.0, 0.5, 0.25)


def _pin_act_table():
    """Force Exp and Ln to resolve to natural_log_exp_and_others so the
    table chooser emits one ACT_TABLE_LOAD instead of thrashing."""
    import concourse.bacc as _bacc
    import concourse.hw_specs as _hw

    if getattr(_bacc, "_act_tables_pinned", False):
        return
    orig = _hw.get_activation_tables

    def patched(arch):
        tabs = orig(arch)
        for name, fns in tabs.items():
            if name != "natural_log_exp_and_others":
                fns.discard(AFT.Exp)
                fns.discard(AFT.Ln)
        return tabs

    _bacc.get_activation_tables = patched
    _bacc._act_tables_pinned = True


def build_module():
    _pin_act_table()
    nc = bacc.Bacc(
        "TRN2",
        target_bir_lowering=False,
        debug=False,
        num_devices=N_CORES,
    )

    out0 = nc.declare_dram_parameter("out0", [B_LOCAL, C, 512, 512], BF16, False)
    out1 = nc.declare_dram_parameter("out1", [B_LOCAL, C, 256, 256], BF16, False)
    out2 = nc.declare_dram_parameter("out2", [B_LOCAL, C, 128, 128], BF16, False)
    tgt = nc.declare_dram_parameter("target", [B_LOCAL, C, H, W], BF16, False)
    loss_out = nc.declare_dram_parameter("loss", [128, 3, 128], F32, isOutput=True)

    # matmuls per scale (for PSUM start/stop flags):
    #   s0: 2 groups * 2 products * 16 chunks = 64
    #   s1: 1 group * 2 products * 8 chunks  = 16
    #   s2: 1 group * 2 products * 2 chunks  = 4
    N_MM_SCALE = [64, 16, 4]
    mm_cnt = [0, 0, 0]

    with ExitStack() as ctx:
        tc = ctx.enter_context(tile.TileContext(nc))
        work = ctx.enter_context(tc.tile_pool(name="work", bufs=2))
        tpool = ctx.enter_context(tc.tile_pool(name="tpool", bufs=1))
        accp = ctx.enter_context(tc.tile_pool(name="accp", bufs=1))
        psum = ctx.enter_context(tc.tile_pool(name="psum", bufs=1, space="PSUM"))

        acc_all = psum.tile([128, 3, 128], F32, tag="acc", name="acc")
        acc_ps = [acc_all[:, s, :] for s in range(3)]

        def mm(scale, t_chunk, a_chunk):
            i = mm_cnt[scale]
            nc.tensor.matmul(
                acc_ps[scale], t_chunk, a_chunk,
                start=(i == 0), stop=(i == N_MM_SCALE[scale] - 1),
            )
            mm_cnt[scale] = i + 1

        # ---------- input DMAs ----------
        x1_t = {}
        for b in range(B_LOCAL):
            x1_t[b] = work.tile([128, 2, 512], BF16, tag=f"x1_{b}", name=f"x1_{b}")
            nc.sync.dma_start(
                out=x1_t[b][:],
                in_=out1[b].rearrange("c (p f) w -> p c (f w)", f=2),
            )
        x0_t, t_t = {}, {}
        x0_t[0] = work.tile([128, 2, 2048], BF16, tag="x0", name="x0_0")
        nc.sync.dma_start(
            out=x0_t[0][:],
            in_=out0[0].rearrange("c (p f) w -> p c (f w)", f=4),
        )
        t_t[0] = tpool.tile([128, 2, 2048], BF16, tag="t_0", name="t_0")
        nc.sync.dma_start(
            out=t_t[0][:],
            in_=tgt[0].rearrange("c (p f) w -> p c (f w)", f=4),
        )
        x0_t[1] = work.tile([128, 2, 2048], BF16, tag="x0", name="x0_1")
        nc.sync.dma_start(
            out=x0_t[1][:],
            in_=out0[1].rearrange("c (p f) w -> p c (f w)", f=4),
        )
        t_t[1] = tpool.tile([128, 2, 2048], BF16, tag="t_1", name="t_1")
        nc.sync.dma_start(
            out=t_t[1][:],
            in_=tgt[1].rearrange("c (p f) w -> p c (f w)", f=4),
        )
        x2_t = work.tile([128, 2, 2, 128], BF16, tag="x2")
        nc.sync.dma_start(
            out=x2_t[:],
            in_=out2.rearrange("b c p w -> p b c w"),
        )

        # ---------- group definitions ----------
        def grp_s1m():
            def d_maker(d_t):
                nc.vector.tensor_sub(d_t[:, 0:512], x1_t[0][:, 1], x1_t[0][:, 0])
                nc.vector.tensor_sub(d_t[:, 512:1024], x1_t[1][:, 1], x1_t[1][:, 0])

            def chunks(c):
                out = []
                for b in range(B_LOCAL):
                    tv = t_t[b][:, c].rearrange("p (r w) -> p r w", r=4)
                    for k in range(4):
                        l, j = k // 2, k % 2
                        out.append(tv[:, 2 * l, slice(256 * j, 256 * j + 256, 2)])
                return out

            return ("1m", 1024, d_maker, 1, chunks)

        def grp_s0(b):
            def d_maker(d_t):
                nc.vector.tensor_sub(d_t[:], x0_t[b][:, 1], x0_t[b][:, 0])

            def chunks(c):
                return [
                    t_t[b][:, c, 128 * k : 128 * (k + 1)] for k in range(16)
                ]

            return (f"0{b}", 2048, d_maker, 0, chunks)

        def grp_s2():
            def d_maker(d_t):
                nc.vector.tensor_sub(
                    d_t[:].rearrange("p (b w) -> p b w", b=2),
                    x2_t[:, :, 1, :],
                    x2_t[:, :, 0, :],
                )

            def chunks(c):
                return [
                    t_t[b][:, c].rearrange("p (r w) -> p r w", r=4)[
                        :, 0, slice(0, 512, 4)
                    ]
                    for b in range(B_LOCAL)
                ]

            return ("2", 256, d_maker, 2, chunks)

        groups = [grp_s1m(), grp_s0(0), grp_s0(1), grp_s2()]

        # ---------- software-pipelined emission ----------
        stageA_out = {}
        stageB_out = {}

        def emit_A(gi):
            key, F, d_maker, scale, chunks = groups[gi]
            d_t = work.tile([128, F], BF16, tag=f"d_{scale}", name=f"d{key}")
            d_maker(d_t)
            stageA_out[gi] = d_t

        def emit_B(gi):
            key, F, d_maker, scale, chunks = groups[gi]
            d_t = stageA_out[gi]
            u_t = work.tile([128, F], BF16, tag=f"u_{scale}", name=f"u{key}")
            nc.scalar.activation(u_t[:], d_t[:], AFT.Exp)
            sp_t = work.tile([128, F], BF16, tag=f"sp_{scale}", name=f"sp{key}")
            nc.scalar.activation(sp_t[:], u_t[:], AFT.Ln, bias=1.0)
            v_t = work.tile([128, F], BF16, tag=f"v_{scale}", name=f"v{key}")
            nc.scalar.activation(v_t[:], sp_t[:], AFT.Exp, scale=-1.0)
            stageB_out[gi] = (d_t, sp_t, v_t)

        def emit_C(gi):
            key, F, d_maker, scale, chunks = groups[gi]
            d_t, sp_t, v_t = stageB_out[gi]
            spm_t = work.tile([128, F], BF16, tag=f"spm_{scale}", name=f"spm{key}")
            nc.vector.tensor_sub(spm_t[:], sp_t[:], d_t[:])
            g_t = work.tile([128, F], BF16, tag=f"g_{scale}", name=f"g{key}")
            nc.vector.tensor_scalar(g_t[:], v_t[:], -1.0, 1.0, ALU.mult, ALU.add)
            om2_t = work.tile([128, F], BF16, tag=f"om2_{scale}", name=f"om2{key}")
            nc.vector.tensor_mul(om2_t[:], v_t[:], v_t[:])
            s2_t = work.tile([128, F], BF16, tag=f"s2_{scale}", name=f"s2{key}")
            nc.vector.tensor_mul(s2_t[:], g_t[:], g_t[:])
            ap_t = work.tile([128, F], BF16, tag=f"ap_{scale}", name=f"ap{key}")
            nc.vector.tensor_mul(ap_t[:], sp_t[:], s2_t[:])
            am_t = work.tile([128, F], BF16, tag=f"am_{scale}", name=f"am{key}")
            nc.vector.tensor_mul(am_t[:], spm_t[:], om2_t[:])
            for c, a_t in ((0, ap_t), (1, am_t)):
                for k, tch in enumerate(chunks(c)):
                    mm(scale, tch, a_t[:, 128 * k : 128 * (k + 1)])

        # pipeline: A0 B0 A1 B1 C0 A2 B2 C1 A3 B3 C2 C3
        emit_A(0)
        emit_B(0)
        emit_A(1)
        emit_B(1)
        emit_C(0)
        emit_A(2)
        emit_B(2)
        emit_C(1)
        emit_A(3)
        emit_B(3)
        emit_C(2)
        emit_C(3)

        assert mm_cnt == N_MM_SCALE, mm_cnt

        # ---------- tail ----------
        red_sb = accp.tile([128, 3, 128], F32, tag="red_sb")
        nc.vector.tensor_copy(red_sb[:], acc_all[:])
        nc.sync.dma_start(out=loss_out[:, :, :], in_=red_sb[:])

    nc.compile()
    return nc


_CACHED_NC = None


def _get_module():
    global _CACHED_NC
    if _CACHED_NC is None:
        _CACHED_NC = build_module()
    return _CACHED_NC


USE_ALLREDUCE = False  # partials summed on host


def make_in_maps(inputs):
    """Shard batch across cores and cast to the device dtypes (bf16)."""
    in_maps = []
    for core in range(N_CORES):
        lo, hi = core * B_LOCAL, (core + 1) * B_LOCAL
        in_maps.append(
            {
                name: np.ascontiguousarray(
                    np.asarray(inputs[name][lo:hi], dtype=np.float32)
                ).astype(np.float16)
                for name in ("out0", "out1", "out2", "target")
            }
        )
    return in_maps


def finalize(results):
    tot = 0.0
    for r in results:
        acc = np.asarray(r["loss"], dtype=np.float64)  # [128, 3, 128]
        for s, wt in enumerate(SCALE_WTS):
            tot += wt * np.trace(acc[:, s, :])
    return np.asarray(np.float32(tot)).reshape(())


def kernel(**inputs) -> np.ndarray:
    nc = _get_module()
    res = run_bass_kernel_spmd(nc, make_in_maps(inputs), list(range(N_CORES)))
    return finalize(res.results)


# revision 9
# speedup vs baseline: 1.0558x; 1.0558x over previous
"""v7: DVE+ACT only (gpsimd contends with DVE for SBUF ports - avoid),
per-scale PSUM accumulators, software-pipelined emission.

Math per pixel: d = x1-x0, u = Exp(d), sp = Ln(u+1), spm = sp-d,
v = Exp(-sp), g = 1-v, s2 = g*g, om2 = v*v,
loss = sum_s wt_s * (t0*sp*s2 + t1*spm*om2)   (wt applied on host).

Lessons baked in (v5/v6 traces):
- gpsimd TENSOR_TENSOR not only runs ~2ns/elem on bf16, it also slows
  concurrent DVE ops 2-3x (shared SBUF ports) -> all elementwise math
  on DVE (tensor_tensor 0.6ns/elem) + ACT (0.94ns/elem).
- STT runs at 1x; with per-scale PSUM accs the wt moves to the host
  and ap/am become 2x tensor_tensor.
- 4 groups: s1-merged [128,1024], s0 per sample [128,2048] x2,
  s2 [128,256] last (short tail). d-subs emitted interleaved so no
  engine head-of-line blocks on a late DMA.
- DMA order: small logits, x0_0, t_0, x0_1, t_1, x2 - targets are only
  needed by the PE masks, which run late anyway.
"""

import os
from contextlib import ExitStack

import numpy as np
import ml_dtypes

import concourse.bacc as bacc
import concourse.bass as bass
import concourse.mybir as mybir
import concourse.tile as tile
from concourse.bass_utils import run_bass_kernel_spmd

F32 = mybir.dt.float32
BF16 = mybir.dt.bfloat16
AFT = mybir.ActivationFunctionType
ALU = mybir.AluOpType

N_CORES = 8
B, C, H, W = 16, 2, 512, 512
B_LOCAL = B // N_CORES  # 2
SCALE_WTS = (1.0, 0.5, 0.25)


def _pin_act_table():
    """Force Exp and Ln to resolve to natural_log_exp_and_others so the
    table chooser emits one ACT_TABLE_LOAD instead of thrashing."""
    import concourse.bacc as _bacc
    import concourse.hw_specs as _hw

    if getattr(_bacc, "_act_tables_pinned", False):
        return
    orig = _hw.get_activation_tables

    def patched(arch):
        tabs = orig(arch)
        for name, fns in tabs.items():
            if name != "natural_log_exp_and_others":
                fns.discard(AFT.Exp)
                fns.discard(AFT.Ln)
        return tabs

    _bacc.get_activation_tables = patched
    _bacc._act_tables_pinned = True


def build_module():
    _pin_act_table()
    nc = bacc.Bacc(
        "TRN2",
        target_bir_lowering=False,
        debug=False,
        num_devices=N_CORES,
    )

    out0 = nc.declare_dram_parameter("out0", [B_LOCAL, C, 512, 512], BF16, False)
    out1 = nc.declare_dram_parameter("out1", [B_LOCAL, C, 256, 256], BF16, False)
    out2 = nc.declare_dram_parameter("out2", [B_LOCAL, C, 128, 128], BF16, False)
    tgt = nc.declare_dram_parameter("target", [B_LOCAL, C, H, W], BF16, False)
    loss_out = nc.declare_dram_parameter("loss", [128, 3, 128], F32, isOutput=True)

    # matmuls per scale (for PSUM start/stop flags):
    #   s0: 2 groups * 2 products * 16 chunks = 64
    #   s1: 1 group * 2 products * 8 chunks  = 16
    #   s2: 1 group * 2 products * 2 chunks  = 4
    N_MM_SCALE = [64, 16, 4]
    mm_cnt = [0, 0, 0]

    with ExitStack() as ctx:
        tc = ctx.enter_context(tile.TileContext(nc))
        work = ctx.enter_context(tc.tile_pool(name="work", bufs=2))
        tpool = ctx.enter_context(tc.tile_pool(name="tpool", bufs=1))
        accp = ctx.enter_context(tc.tile_pool(name="accp", bufs=1))
        psum = ctx.enter_context(tc.tile_pool(name="psum", bufs=1, space="PSUM"))

        acc_all = psum.tile([128, 3, 128], F32, tag="acc", name="acc")
        acc_ps = [acc_all[:, s, :] for s in range(3)]

        def mm(scale, t_chunk, a_chunk):
            i = mm_cnt[scale]
            nc.tensor.matmul(
                acc_ps[scale], t_chunk, a_chunk,
                start=(i == 0), stop=(i == N_MM_SCALE[scale] - 1),
            )
            mm_cnt[scale] = i + 1

        # ---------- input DMAs ----------
        x1_t = {}
        for b in range(B_LOCAL):
            x1_t[b] = work.tile([128, 2, 512], BF16, tag=f"x1_{b}", name=f"x1_{b}")
            nc.sync.dma_start(
                out=x1_t[b][:],
                in_=out1[b].rearrange("c (p f) w -> p c (f w)", f=2),
            )
        x0_t, t_t = {}, {}
        x0_t[0] = work.tile([128, 2, 2048], BF16, tag="x0", name="x0_0")
        nc.sync.dma_start(
            out=x0_t[0][:],
            in_=out0[0].rearrange("c (p f) w -> p c (f w)", f=4),
        )
        t_t[0] = tpool.tile([128, 2, 2048], BF16, tag="t_0", name="t_0")
        nc.sync.dma_start(
            out=t_t[0][:],
            in_=tgt[0].rearrange("c (p f) w -> p c (f w)", f=4),
        )
        x0_t[1] = work.tile([128, 2, 2048], BF16, tag="x0", name="x0_1")
        nc.sync.dma_start(
            out=x0_t[1][:],
            in_=out0[1].rearrange("c (p f) w -> p c (f w)", f=4),
        )
        t_t[1] = tpool.tile([128, 2, 2048], BF16, tag="t_1", name="t_1")
        nc.sync.dma_start(
            out=t_t[1][:],
            in_=tgt[1].rearrange("c (p f) w -> p c (f w)", f=4),
        )
        x2_t = work.tile([128, 2, 2, 128], BF16, tag="x2")
        nc.sync.dma_start(
            out=x2_t[:],
            in_=out2.rearrange("b c p w -> p b c w"),
        )

        # ---------- group definitions ----------
        def grp_s1m():
            def d_maker(d_t):
                nc.vector.tensor_sub(d_t[:, 0:512], x1_t[0][:, 1], x1_t[0][:, 0])
                nc.vector.tensor_sub(d_t[:, 512:1024], x1_t[1][:, 1], x1_t[1][:, 0])

            def chunks(c):
                out = []
                for b in range(B_LOCAL):
                    tv = t_t[b][:, c].rearrange("p (r w) -> p r w", r=4)
                    for k in range(4):
                        l, j = k // 2, k % 2
                        out.append(tv[:, 2 * l, slice(256 * j, 256 * j + 256, 2)])
                return out

            return ("1m", 1024, d_maker, 1, chunks)

        def grp_s0(b):
            def d_maker(d_t):
                nc.vector.tensor_sub(d_t[:], x0_t[b][:, 1], x0_t[b][:, 0])

            def chunks(c):
                return [
                    t_t[b][:, c, 128 * k : 128 * (k + 1)] for k in range(16)
                ]

            return (f"0{b}", 2048, d_maker, 0, chunks)

        def grp_s2():
            def d_maker(d_t):
                nc.vector.tensor_sub(
                    d_t[:].rearrange("p (b w) -> p b w", b=2),
                    x2_t[:, :, 1, :],
                    x2_t[:, :, 0, :],
                )

            def chunks(c):
                return [
                    t_t[b][:, c].rearrange("p (r w) -> p r w", r=4)[
                        :, 0, slice(0, 512, 4)
                    ]
                    for b in range(B_LOCAL)
                ]

            return ("2", 256, d_maker, 2, chunks)

        groups = [grp_s1m(), grp_s0(0), grp_s0(1), grp_s2()]

        # ---------- software-pipelined emission ----------
        stageA_out = {}
        stageB_out = {}

        def emit_A(gi):
            key, F, d_maker, scale, chunks = groups[gi]
            d_t = work.tile([128, F], BF16, tag=f"d_{scale}", name=f"d{key}")
            d_maker(d_t)
            stageA_out[gi] = d_t

        def emit_B(gi):
            key, F, d_maker, scale, chunks = groups[gi]
            d_t = stageA_out[gi]
            u_t = work.tile([128, F], BF16, tag=f"u_{scale}", name=f"u{key}")
            nc.scalar.activation(u_t[:], d_t[:], AFT.Exp)
            sp_t = work.tile([128, F], BF16, tag=f"sp_{scale}", name=f"sp{key}")
            nc.scalar.activation(sp_t[:], u_t[:], AFT.Ln, bias=1.0)
            v_t = work.tile([128, F], BF16, tag=f"v_{scale}", name=f"v{key}")
            nc.scalar.activation(v_t[:], sp_t[:], AFT.Exp, scale=-1.0)
            stageB_out[gi] = (d_t, sp_t, v_t)

        def emit_C(gi):
            key, F, d_maker, scale, chunks = groups[gi]
            d_t, sp_t, v_t = stageB_out[gi]
            spm_t = work.tile([128, F], BF16, tag=f"spm_{scale}", name=f"spm{key}")
            nc.vector.tensor_sub(spm_t[:], sp_t[:], d_t[:])
            g_t = work.tile([128, F], BF16, tag=f"g_{scale}", name=f"g{key}")
            nc.vector.tensor_scalar(g_t[:], v_t[:], -1.0, 1.0, ALU.mult, ALU.add)
            om2_t = work.tile([128, F], BF16, tag=f"om2_{scale}", name=f"om2{key}")
            nc.vector.tensor_mul(om2_t[:], v_t[:], v_t[:])
            s2_t = work.tile([128, F], BF16, tag=f"s2_{scale}", name=f"s2{key}")
            nc.vector.tensor_mul(s2_t[:], g_t[:], g_t[:])
            ap_t = work.tile([128, F], BF16, tag=f"ap_{scale}", name=f"ap{key}")
            nc.vector.tensor_mul(ap_t[:], sp_t[:], s2_t[:])
            am_t = work.tile([128, F], BF16, tag=f"am_{scale}", name=f"am{key}")
            nc.vector.tensor_mul(am_t[:], spm_t[:], om2_t[:])
            for c, a_t in ((0, ap_t), (1, am_t)):
                for k, tch in enumerate(chunks(c)):
                    mm(scale, tch, a_t[:, 128 * k : 128 * (k + 1)])

        # pipeline: A0 B0 A1 B1 C0 A2 B2 C1 A3 B3 C2 C3
        emit_A(0)
        emit_B(0)
        emit_A(1)
        emit_B(1)
        emit_C(0)
        emit_A(2)
        emit_B(2)
        emit_C(1)
        emit_A(3)
        emit_B(3)
        emit_C(2)
        emit_C(3)

        assert mm_cnt == N_MM_SCALE, mm_cnt

        # ---------- tail ----------
        red_sb = accp.tile([128, 3, 128], F32, tag="red_sb")
        nc.vector.tensor_copy(red_sb[:], acc_all[:])
        nc.sync.dma_start(out=loss_out[:, :, :], in_=red_sb[:])

    nc.compile()
    return nc


_CACHED_NC = None


def _get_module():
    global _CACHED_NC
    if _CACHED_NC is None:
        _CACHED_NC = build_module()
    return _CACHED_NC


USE_ALLREDUCE = False  # partials summed on host


def make_in_maps(inputs):
    """Shard batch across cores and cast to the device dtypes (bf16)."""
    in_maps = []
    for core in range(N_CORES):
        lo, hi = core * B_LOCAL, (core + 1) * B_LOCAL
        in_maps.append(
            {
                name: np.ascontiguousarray(
                    np.asarray(inputs[name][lo:hi], dtype=np.float32)
                ).astype(ml_dtypes.bfloat16)
                for name in ("out0", "out1", "out2", "target")
            }
        )
    return in_maps


def finalize(results):
    tot = 0.0
    for r in results:
        acc = np.asarray(r["loss"], dtype=np.float64)  # [128, 3, 128]
        for s, wt in enumerate(SCALE_WTS):
            tot += wt * np.trace(acc[:, s, :])
    return np.asarray(np.float32(tot)).reshape(())


def kernel(**inputs) -> np.ndarray:
    nc = _get_module()
    res = run_bass_kernel_spmd(nc, make_in_maps(inputs), list(range(N_CORES)))
    return finalize(res.results)


# revision 13
# speedup vs baseline: 1.1811x; 1.1187x over previous
"""v9: host-packed small inputs (one 5KB-line DMA), split tail group,
hand-tuned software-pipelined emission. DVE+ACT only.

Math per pixel: d = x1-x0, u = Exp(d), sp = Ln(u+1), spm = sp-d,
v = Exp(-sp), g = 1-v, s2 = g*g, om2 = v*v,
loss = sum_s wt_s * (t0*sp*s2 + t1*spm*om2)   (wt applied on host).

Trace-driven lessons:
- DMA packets cost ~180ns/packet/engine regardless of size, so the
  scale-1/2 logits (1KB / 256B lines) wasted ~9us of ring time. The
  host now packs them into one [128, 2560] array -> 5KB contiguous
  lines, 128 packets, ~1.2us.
- gpsimd contends with DVE for SBUF ports (v6) - unused.
- scalar-engine DMA issue consistently regressed - all DMAs on sync.
- fp16 is slower than bf16 on both ACT and DVE (conversion penalty).
- One PSUM accumulator per scale (wt on host) keeps products on 2x
  tensor_tensor; separate PSUM tiles beat slices of one tile.
- Last scale-0 sample split into two half groups (shorter drain).
- Group/DMA order: s1m leads (small, lands first), then s0 samples,
  s2 last; s1m's t_1-masked matmuls are deferred behind s0b0's so the
  PE queue never head-of-line blocks on the late t_1 DMA.
"""

import os
from contextlib import ExitStack

import numpy as np
import ml_dtypes

import concourse.bacc as bacc
import concourse.bass as bass
import concourse.mybir as mybir
import concourse.tile as tile
from concourse.bass_utils import run_bass_kernel_spmd

F32 = mybir.dt.float32
BF16 = mybir.dt.bfloat16
AFT = mybir.ActivationFunctionType
ALU = mybir.AluOpType

N_CORES = 8
B, C, H, W = 16, 2, 512, 512
B_LOCAL = B // N_CORES  # 2
SCALE_WTS = (1.0, 0.5, 0.25)


def _pin_act_table():
    """Force Exp and Ln to resolve to natural_log_exp_and_others so the
    table chooser emits one ACT_TABLE_LOAD instead of thrashing."""
    import concourse.bacc as _bacc
    import concourse.hw_specs as _hw

    if getattr(_bacc, "_act_tables_pinned", False):
        return
    orig = _hw.get_activation_tables

    def patched(arch):
        tabs = orig(arch)
        for name, fns in tabs.items():
            if name != "natural_log_exp_and_others":
                fns.discard(AFT.Exp)
                fns.discard(AFT.Ln)
        return tabs

    _bacc.get_activation_tables = patched
    _bacc._act_tables_pinned = True


def build_module():
    _pin_act_table()
    nc = bacc.Bacc(
        "TRN2",
        target_bir_lowering=False,
        debug=False,
        num_devices=N_CORES,
    )

    out0 = nc.declare_dram_parameter("out0", [B_LOCAL, C, 512, 512], BF16, False)
    # x12: host-packed scale-1 + scale-2 logits:
    #   [:, 0:2048]    = out1 as [p][b, c, l*256+w], label rows 2p+l
    #   [:, 2048:2560] = out2 as [p][b, c, w], label row p
    x12 = nc.declare_dram_parameter("x12", [128, 2560], BF16, False)
    tgt = nc.declare_dram_parameter("target", [B_LOCAL, C, H, W], BF16, False)
    loss_out = nc.declare_dram_parameter("loss", [128, 3, 128], F32, isOutput=True)

    # matmuls per scale (start/stop by emission order within each scale)
    N_MM_SCALE = [64, 16, 4]
    mm_cnt = [0, 0, 0]

    with ExitStack() as ctx:
        tc = ctx.enter_context(tile.TileContext(nc))
        once = ctx.enter_context(tc.tile_pool(name="once", bufs=1))
        psum = ctx.enter_context(tc.tile_pool(name="psum", bufs=1, space="PSUM"))

        acc_ps = [psum.tile([128, 128], F32, tag=f"acc{s}", name=f"acc{s}")
                  for s in range(3)]

        def mm(scale, t_chunk, a_chunk):
            i = mm_cnt[scale]
            nc.tensor.matmul(
                acc_ps[scale][:], t_chunk, a_chunk,
                start=(i == 0), stop=(i == N_MM_SCALE[scale] - 1),
            )
            mm_cnt[scale] = i + 1

        # ---------- input DMAs (all on the sync ring, in order) ----------
        x12_t = once.tile([128, 2560], BF16, tag="x12", name="x12_t")
        nc.sync.dma_start(out=x12_t[:], in_=x12[:, :])
        x0_t = {}
        x0_t[0] = once.tile([128, 2, 2048], BF16, tag="x0_0", name="x0_0")
        nc.sync.dma_start(
            out=x0_t[0][:],
            in_=out0[0].rearrange("c (p f) w -> p c (f w)", f=4),
        )
        x01h = {}
        for h in range(2):
            x01h[h] = once.tile([128, 2, 1024], BF16, tag=f"x01{h}", name=f"x01{h}")
            nc.sync.dma_start(
                out=x01h[h][:],
                in_=out0[1].rearrange("c (p f) w -> p c (f w)", f=4)[
                    :, :, 1024 * h : 1024 * (h + 1)
                ],
            )
        t_t = {}
        t_t[0] = once.tile([128, 2, 2048], BF16, tag="t_0", name="t_0")
        nc.sync.dma_start(
            out=t_t[0][:],
            in_=tgt[0].rearrange("c (p f) w -> p c (f w)", f=4),
        )
        t_t[1] = once.tile([128, 2, 2048], BF16, tag="t_1", name="t_1")
        nc.sync.dma_start(
            out=t_t[1][:],
            in_=tgt[1].rearrange("c (p f) w -> p c (f w)", f=4),
        )

        # ---------- group definitions ----------
        def s1_chunks(c):
            out = []
            for b in range(B_LOCAL):
                tv = t_t[b][:, c].rearrange("p (r w) -> p r w", r=4)
                for k in range(4):
                    l, j = k // 2, k % 2
                    out.append(tv[:, 2 * l, slice(256 * j, 256 * j + 256, 2)])
            return out

        def grp_s1m():
            x12v = x12_t[:, 0:2048].rearrange("p (b c fw) -> p b c fw", b=2, c=2)

            def d_maker(d_t):
                nc.vector.tensor_sub(d_t[:, 0:512], x12v[:, 0, 1], x12v[:, 0, 0])
                nc.vector.tensor_sub(d_t[:, 512:1024], x12v[:, 1, 1], x12v[:, 1, 0])

            return ("1m", 1024, d_maker, 1, s1_chunks)

        def grp_s00():
            def d_maker(d_t):
                nc.vector.tensor_sub(d_t[:], x0_t[0][:, 1], x0_t[0][:, 0])

            def chunks(c):
                return [t_t[0][:, c, 128 * k : 128 * (k + 1)] for k in range(16)]

            return ("00", 2048, d_maker, 0, chunks)

        def grp_s01(h):
            def d_maker(d_t):
                nc.vector.tensor_sub(d_t[:], x01h[h][:, 1], x01h[h][:, 0])

            def chunks(c):
                base = 1024 * h
                return [
                    t_t[1][:, c, base + 128 * k : base + 128 * (k + 1)]
                    for k in range(8)
                ]

            return (f"01{h}", 1024, d_maker, 0, chunks)

        def grp_s2():
            x2v = x12_t[:, 2048:2560].rearrange("p (b c w) -> p b c w", b=2, c=2)

            def d_maker(d_t):
                nc.vector.tensor_sub(
                    d_t[:].rearrange("p (b w) -> p b w", b=2),
                    x2v[:, :, 1, :],
                    x2v[:, :, 0, :],
                )

            def chunks(c):
                return [
                    t_t[b][:, c].rearrange("p (r w) -> p r w", r=4)[
                        :, 0, slice(0, 512, 4)
                    ]
                    for b in range(B_LOCAL)
                ]

            return ("2", 256, d_maker, 2, chunks)

        groups = [grp_s1m(), grp_s00(), grp_s01(0), grp_s01(1), grp_s2()]
        G1M, G00, G01A, G01B, G2 = range(5)

        stageA_out = {}
        stageB_out = {}

        def emit_A(gi):
            key, F, d_maker, scale, chunks = groups[gi]
            d_t = once.tile([128, F], BF16, tag=f"d{key}", name=f"d{key}")
            d_maker(d_t)
            stageA_out[gi] = d_t

        def emit_B(gi):
            key, F, d_maker, scale, chunks = groups[gi]
            d_t = stageA_out[gi]
            u_t = once.tile([128, F], BF16, tag=f"u{key}", name=f"u{key}")
            nc.scalar.activation(u_t[:], d_t[:], AFT.Exp)
            sp_t = once.tile([128, F], BF16, tag=f"sp{key}", name=f"sp{key}")
            nc.scalar.activation(sp_t[:], u_t[:], AFT.Ln, bias=1.0)
            v_t = once.tile([128, F], BF16, tag=f"v{key}", name=f"v{key}")
            nc.scalar.activation(v_t[:], sp_t[:], AFT.Exp, scale=-1.0)
            stageB_out[gi] = (d_t, sp_t, v_t)

        mm_pending = {}

        def emit_mms(gi, ap_t, am_t, mm_sel):
            key, F, d_maker, scale, chunks = groups[gi]
            for c, a_t in ((0, ap_t), (1, am_t)):
                for k, tch in enumerate(chunks(c)):
                    if mm_sel is not None and not mm_sel(c, k):
                        mm_pending.setdefault(gi, []).append(
                            (scale, tch, a_t[:, 128 * k : 128 * (k + 1)])
                        )
                    else:
                        mm(scale, tch, a_t[:, 128 * k : 128 * (k + 1)])

        def emit_deferred(gi):
            for scale, tch, ach in mm_pending.pop(gi, []):
                mm(scale, tch, ach)

        def emit_C(gi, mm_sel=None):
            key, F, d_maker, scale, chunks = groups[gi]
            d_t, sp_t, v_t = stageB_out[gi]
            spm_t = once.tile([128, F], BF16, tag=f"spm{key}", name=f"spm{key}")
            nc.vector.tensor_sub(spm_t[:], sp_t[:], d_t[:])
            g_t = once.tile([128, F], BF16, tag=f"g{key}", name=f"g{key}")
            nc.vector.tensor_scalar(g_t[:], v_t[:], -1.0, 1.0, ALU.mult, ALU.add)
            om2_t = once.tile([128, F], BF16, tag=f"om2{key}", name=f"om2{key}")
            nc.vector.tensor_mul(om2_t[:], v_t[:], v_t[:])
            s2_t = once.tile([128, F], BF16, tag=f"s2{key}", name=f"s2{key}")
            nc.vector.tensor_mul(s2_t[:], g_t[:], g_t[:])
            ap_t = once.tile([128, F], BF16, tag=f"ap{key}", name=f"ap{key}")
            nc.vector.tensor_mul(ap_t[:], sp_t[:], s2_t[:])
            am_t = once.tile([128, F], BF16, tag=f"am{key}", name=f"am{key}")
            nc.vector.tensor_mul(am_t[:], spm_t[:], om2_t[:])
            emit_mms(gi, ap_t, am_t, mm_sel)

        # ---------- emission schedule ----------
        emit_A(G1M)
        emit_B(G1M)
        emit_A(G00)
        emit_B(G00)
        # s1m products; only its t_0-masked matmuls now (t_1 lands late)
        emit_C(G1M, mm_sel=lambda c, k: k < 4)
        emit_A(G01A)
        emit_B(G01A)
        emit_C(G00)
        emit_deferred(G1M)
        emit_A(G01B)
        emit_B(G01B)
        emit_A(G2)
        emit_C(G01A)
        emit_B(G2)
        emit_C(G01B)
        emit_C(G2)

        assert mm_cnt == N_MM_SCALE, mm_cnt

        # ---------- tail ----------
        red_sb = once.tile([128, 3, 128], F32, tag="red_sb")
        for s in range(3):
            nc.vector.tensor_copy(red_sb[:, s, :], acc_ps[s][:])
        nc.sync.dma_start(out=loss_out[:, :, :], in_=red_sb[:])

    nc.compile()
    return nc


_CACHED_NC = None


def _get_module():
    global _CACHED_NC
    if _CACHED_NC is None:
        _CACHED_NC = build_module()
    return _CACHED_NC


USE_ALLREDUCE = False  # partials summed on host


def make_in_maps(inputs):
    """Shard batch across cores, cast to bf16, pack scale-1/2 logits."""
    bf = ml_dtypes.bfloat16
    in_maps = []
    out1 = np.asarray(inputs["out1"], dtype=np.float32)
    out2 = np.asarray(inputs["out2"], dtype=np.float32)
    for core in range(N_CORES):
        lo, hi = core * B_LOCAL, (core + 1) * B_LOCAL
        # x12[p] = [out1 as (b, c, l, w) with label row 2p+l | out2 row p]
        a1 = out1[lo:hi].reshape(B_LOCAL, C, 128, 2, 256)
        a1 = a1.transpose(2, 0, 1, 3, 4).reshape(128, 2048)
        a2 = out2[lo:hi].transpose(2, 0, 1, 3).reshape(128, 512)
        x12_arr = np.concatenate([a1, a2], axis=1).astype(bf)
        in_maps.append(
            {
                "out0": np.ascontiguousarray(
                    np.asarray(inputs["out0"][lo:hi], dtype=np.float32)
                ).astype(bf),
                "x12": np.ascontiguousarray(x12_arr),
                "target": np.ascontiguousarray(
                    np.asarray(inputs["target"][lo:hi], dtype=np.float32)
                ).astype(bf),
            }
        )
    return in_maps


def finalize(results):
    tot = 0.0
    for r in results:
        acc = np.asarray(r["loss"], dtype=np.float64)  # [128, 3, 128]
        for s, wt in enumerate(SCALE_WTS):
            tot += wt * np.trace(acc[:, s, :])
    return np.asarray(np.float32(tot)).reshape(())


def kernel(**inputs) -> np.ndarray:
    nc = _get_module()
    res = run_bass_kernel_spmd(nc, make_in_maps(inputs), list(range(N_CORES)))
    return finalize(res.results)
